# revision 12
# baseline (speedup 1.0000x reference)
"""MoE actor kernel for 8 TRN2 NeuronCores (expert-parallel, host routing).

Problem: B=65536 tokens, obs dim D=376, each routed by `o` to one of E=8
experts; per-expert MLP 376 -> 256 -> 256 -> {mean[17], log_std[17]} with
relu/relu/(identity|tanh-affine) heads.

Strategy: routing/gather happens on the host (numpy) — core e receives
exactly the tokens assigned to expert e (padded to NPAD, a multiple of 512)
plus only that expert's weights. Every core runs the same dense 3-layer MLP
graph with features on the partition axis:

    h1T[H, n] = relu(W1.T @ xT + b1)     K=384(pad of 376) -> M=256
    h2T[H, n] = relu(W2.T @ h1T + b2)    K=256 -> M=256
    zT[34, n] = Wc.T @ h2T + bc          K=256 -> M=34 (mean ++ log_std)
    out rows 17:34 = 3.5*tanh(z) - 1.5

Matmuls run in float32r (full PE rate at N=512, ~1e-4 rel err). The host
scatters per-core outputs back to the original token order.
"""

import numpy as np

B, D, H, A, E = 65536, 376, 256, 17, 8
DPAD = 384          # D padded to 3 partition tiles of 128
TOK = 512           # token tile (matmul free dim; one PSUM bank)
AOUT = 2 * A        # 34: mean ++ log_std

# test.py hooks: set TRACE=True before calling kernel() to profile; the
# BassKernelResults of the last run lands in LAST_RESULT.
TRACE = False
TRACE_CORES = None
LAST_RESULT = None

_cache = {}


def _install_axon_ntff_hook():
    """antenv.axon_hooks is absent in this image; recreate it so
    run_bass_kernel_spmd(trace=True) can capture NTFF profiles."""
    import sys, types
    if 'antenv.axon_hooks' in sys.modules:
        return
    try:
        from trn_agent_boot.trn_boot import _ntff_profile_via_ctypes
        hook = _ntff_profile_via_ctypes('/opt/axon/libaxon_pjrt.so')
    except Exception:
        hook = None
    m = types.ModuleType('antenv.axon_hooks')
    m.get_axon_ntff_profile_hook = lambda: hook
    m.set_axon_ntff_profile_hook = lambda h: None
    sys.modules['antenv.axon_hooks'] = m


def _build(npad):
    import concourse.bass as bass
    import concourse.tile as tile
    from concourse import bacc, mybir

    f32 = mybir.dt.float32
    f32r = mybir.dt.float32r
    AF = mybir.ActivationFunctionType
    ds = bass.ds
    nt = npad // TOK

    nc = bacc.Bacc("TRN2", target_bir_lowering=False, debug=False, num_devices=E)
    x_ext = nc.dram_tensor("x", [128, nt, 3 * TOK], f32r, kind="ExternalInput")
    w1_ext = nc.dram_tensor("w1", [128, 3 * H], f32r, kind="ExternalInput")
    w2_ext = nc.dram_tensor("w2", [128, 2 * H], f32r, kind="ExternalInput")
    wc_ext = nc.dram_tensor("wc", [128, 2 * 64], f32r, kind="ExternalInput")
    b1_ext = nc.dram_tensor("b1", [128, 2], f32, kind="ExternalInput")
    b2_ext = nc.dram_tensor("b2", [128, 2], f32, kind="ExternalInput")
    bc_ext = nc.dram_tensor("bc", [64, 1], f32, kind="ExternalInput")
    out_ext = nc.dram_tensor("out", [nt, AOUT, TOK], f32, kind="ExternalOutput")

    with tile.TileContext(nc) as tc:
        with tc.tile_pool(name="wp", bufs=1) as wp, \
             tc.tile_pool(name="xp", bufs=4) as xp, \
             tc.tile_pool(name="hp", bufs=2) as hp, \
             tc.tile_pool(name="op", bufs=3) as op, \
             tc.tile_pool(name="ps1", bufs=1, space="PSUM") as ps1, \
             tc.tile_pool(name="ps2", bufs=1, space="PSUM") as ps2, \
             tc.tile_pool(name="ps3", bufs=2, space="PSUM") as ps3:
            # PE pre-warm: dummy matmuls with no input deps keep the HAM
            # activity window busy while the first x tiles stream in, so the
            # real matmul phase starts at 2.4 GHz instead of 1.2.
            warm_w = wp.tile([128, 128], f32, name="warm_w")
            nc.gpsimd.memset(warm_w[:], 0.0)
            warm_x = wp.tile([128, 128], f32, name="warm_x")
            nc.gpsimd.memset(warm_x[:], 0.0)
            pw = ps3.tile([64, TOK], f32, tag="p3", name="pwarm")
            for _ in range(10):
                nc.tensor.matmul(pw[:, 0:128], warm_w[:, 0:64], warm_x[:],
                                 start=True, stop=True)

            w1 = wp.tile([128, 3 * H], f32r)
            nc.gpsimd.dma_start(w1[:], w1_ext.ap()[:])
            w2 = wp.tile([128, 2 * H], f32r)
            nc.gpsimd.dma_start(w2[:], w2_ext.ap()[:])
            wc = wp.tile([128, 2 * 64], f32r)
            nc.gpsimd.dma_start(wc[:], wc_ext.ap()[:])
            b1 = wp.tile([128, 2], f32)
            nc.gpsimd.dma_start(b1[:], b1_ext.ap()[:])
            b2 = wp.tile([128, 2], f32)
            nc.gpsimd.dma_start(b2[:], b2_ext.ap()[:])
            bc = wp.tile([64, 1], f32)
            nc.gpsimd.dma_start(bc[:], bc_ext.ap()[:])

            def head_tail(t, h2):
                # L3 + epilogue for tile t (deferred one iteration so the
                # PE rolls straight into the next tile's L1/L2). Mean rows
                # leave PSUM raw (host adds bm); log_std rows get
                # tanh(z + bs) here and the affine on the host.
                p3 = ps3.tile([64, TOK], f32, tag="p3")
                for k in range(2):
                    nc.tensor.matmul(
                        p3[:], wc[:, ds(k * 64, 64)], h2[k][:],
                        start=(k == 0), stop=(k == 1))
                ot = op.tile([64, TOK], f32, tag="ot")
                nc.vector.tensor_copy(ot[0:A, :], p3[0:A, :])
                nc.scalar.activation(ot[32:32 + A, :], p3[32:32 + A, :], AF.Tanh,
                                     bias=bc[32:32 + A, :])
                nc.gpsimd.dma_start(out_ext.ap()[t, 0:A], ot[0:A, :])
                nc.gpsimd.dma_start(out_ext.ap()[t, A:AOUT], ot[32:32 + A, :])

            prev = None
            for t in range(nt):
                xsb = xp.tile([128, 3 * TOK], f32r, tag="x")
                nc.sync.dma_start(xsb[:], x_ext.ap()[:, t])
                xk = [xsb[:, ds(k * TOK, TOK)] for k in range(3)]

                h1 = []
                for m in range(2):
                    p1 = ps1.tile([128, TOK], f32, tag=f"p1_{m}")
                    for k in range(3):
                        nc.tensor.matmul(
                            p1[:], w1[:, ds(k * H + m * 128, 128)], xk[k],
                            start=(k == 0), stop=(k == 2))
                    h = hp.tile([128, TOK], f32r, tag=f"h1_{m}")
                    if m == 0:
                        nc.scalar.activation(h[:], p1[:], AF.Relu,
                                             bias=b1[:, ds(m, 1)])
                    else:
                        nc.vector.tensor_scalar(
                            out=h[:], in0=p1[:],
                            scalar1=b1[:, ds(m, 1)], scalar2=0.0,
                            op0=mybir.AluOpType.add, op1=mybir.AluOpType.max)
                    h1.append(h)

                # k-major order: the k=0 matmuls only need h1[0], giving the
                # engine producing h1[1] time to finish.
                p2 = [ps2.tile([128, TOK], f32, tag=f"p2_{m}", name=f"p2_{m}")
                      for m in range(2)]
                for k in range(2):
                    for m in range(2):
                        nc.tensor.matmul(
                            p2[m][:], w2[:, ds(k * H + m * 128, 128)], h1[k][:],
                            start=(k == 0), stop=(k == 1))
                h2 = []
                for m in range(2):
                    h = hp.tile([128, TOK], f32r, tag=f"h2_{m}")
                    if m == 0:
                        nc.scalar.activation(h[:], p2[m][:], AF.Relu,
                                             bias=b2[:, ds(m, 1)])
                    else:
                        nc.vector.tensor_scalar(
                            out=h[:], in0=p2[m][:],
                            scalar1=b2[:, ds(m, 1)], scalar2=0.0,
                            op0=mybir.AluOpType.add, op1=mybir.AluOpType.max)
                    h2.append(h)

                if prev is not None:
                    head_tail(prev[0], prev[1])
                prev = (t, h2)
            head_tail(prev[0], prev[1])

    nc.compile()
    return nc


def _get_compiled(npad):
    nc = _cache.get(npad)
    if nc is None:
        nc = _build(npad)
        _cache[npad] = nc
    return nc


def kernel(x, o, W1, b1, W2, b2, Wm, bm, Ws, bs):
    global LAST_RESULT
    from concourse import bass_utils

    x = np.asarray(x, dtype=np.float32)
    o_i = np.asarray(o).astype(np.int64)
    W1 = np.asarray(W1, dtype=np.float32)
    b1 = np.asarray(b1, dtype=np.float32)
    W2 = np.asarray(W2, dtype=np.float32)
    b2 = np.asarray(b2, dtype=np.float32)
    Wm = np.asarray(Wm, dtype=np.float32)
    bm = np.asarray(bm, dtype=np.float32)
    Ws = np.asarray(Ws, dtype=np.float32)
    bs = np.asarray(bs, dtype=np.float32)

    nb, d = x.shape
    counts = np.bincount(o_i, minlength=E)
    npad = max(TOK, int(-(-counts.max() // TOK)) * TOK)
    nt = npad // TOK
    order = np.argsort(o_i, kind="stable")
    idx_per_e = np.split(order, np.cumsum(counts)[:-1])

    in_maps = []
    for e in range(E):
        idx = idx_per_e[e]
        xg = np.zeros((npad, DPAD), np.float32)
        xg[:len(idx), :d] = x[idx]
        x_pack = np.ascontiguousarray(
            xg.reshape(nt, TOK, 3, 128).transpose(3, 0, 2, 1)).reshape(
                128, nt, 3 * TOK)

        w1p = np.zeros((DPAD, H), np.float32)
        w1p[:d] = W1[e]
        w1_pack = np.ascontiguousarray(
            w1p.reshape(3, 128, H).transpose(1, 0, 2)).reshape(128, 3 * H)
        w2_pack = np.ascontiguousarray(
            W2[e].reshape(2, 128, H).transpose(1, 0, 2)).reshape(128, 2 * H)
        wc_full = np.zeros((H, 64), np.float32)
        wc_full[:, 0:A] = Wm[e]
        wc_full[:, 32:32 + A] = Ws[e]
        wc_pack = np.ascontiguousarray(
            wc_full.reshape(2, 128, 64).transpose(1, 0, 2)).reshape(128, 2 * 64)
        b1_pack = np.ascontiguousarray(b1[e].reshape(2, 128).T)
        b2_pack = np.ascontiguousarray(b2[e].reshape(2, 128).T)
        bc_pack = np.zeros((64, 1), np.float32)
        bc_pack[0:A, 0] = bm[e]
        bc_pack[32:32 + A, 0] = bs[e]

        in_maps.append({
            "x": x_pack, "w1": w1_pack, "w2": w2_pack, "wc": wc_pack,
            "b1": b1_pack, "b2": b2_pack, "bc": bc_pack,
        })

    nc = _get_compiled(npad)

    kwargs = {}
    if TRACE:
        _install_axon_ntff_hook()
        bass_utils.upload_artifacts = lambda tmpdir: f"local:{tmpdir}"
        kwargs["trace"] = True
        if TRACE_CORES is not None:
            kwargs["trace_cores"] = TRACE_CORES
    res = bass_utils.run_bass_kernel_spmd(nc, in_maps, core_ids=list(range(E)),
                                          **kwargs)
    LAST_RESULT = res

    mean = np.empty((nb, A), np.float32)
    log_std = np.empty((nb, A), np.float32)
    for e in range(E):
        out = res.results[e]["out"]                       # [nt, 34, TOK]
        ofull = out.transpose(0, 2, 1).reshape(npad, AOUT)
        idx = idx_per_e[e]
        mean[idx] = ofull[:len(idx), :A] + bm[e]
        log_std[idx] = 3.5 * ofull[:len(idx), A:AOUT] - 1.5
    return mean, log_std
